# revision 16
# baseline (speedup 1.0000x reference)
"""Bass/Tile TRN2 kernel for nn_BertSelfAttention2 (B=2, S=2048, D=1024, H=16).

Sharding: 8 cores = 2 (batch) x 4 (head groups of 4 heads). Each core
computes Q/K projections for its 4 heads (2 packed pairs), the modified
attention (kt = softplus(k), v = q + k, mask on the query axis), and
writes its ctx^T slices; the host does the final divide + transpose.

v5 design (query compaction + full PE/ACT software pipeline):
- The mask hits only the QUERY axis; a masked query's softmax is uniform
  so its output is mean_k V[k] -- identical for every masked query of a
  (batch, head). The host permutes positions so unmasked queries come
  first, the device computes attention only for the first Mp (= padded
  unmasked count) query positions, and the host fills masked rows with
  mean(V) = mean_k X[k] @ (Wq+Wk) + bq + bk computed in fp32 on CPU.
  Key-axis work (projections, softplus, V') stays full-length: V = Q + K
  needs Q at every position. For the harness inputs Mp = 1152 of 2048.
- No mask machinery on device: pad queries in [M_b, Mp) are real (masked)
  positions whose outputs the host discards.
- Scores matmuls use 64-row operands (row group h0/h64 per head).
- The ACT engine is the bottleneck once attention starts (~1150ns per
  [128,1024] exp); the schedule keeps it saturated: Kproj(g0) -> te/Ln(g0)
  -> Qproj(g0,c0)+V' -> attention starts at ~t20. ALL remaining
  projection work (Qproj g0 c1-3, Kproj/Qproj/V' of g1) is emitted as
  fill units, one per supertile boundary, inside the attention stream.
  ctx matmuls lag one supertile behind scores so the in-order PE queue
  never stalls waiting for an exp. 5 ACT table loads total
  (Exp | Ln g0 | Exp: attn-g0 + te-g1 | Ln g1 | Exp: attn-g1).
- PSUM tags: sA,sB = [128,1024] f32 scores supertiles (2 banks each,
  double-buffered across supers), cA,cB = ctx accumulators (1 bank),
  qA,qB = projection chunk accumulators + V' transposes (1 bank) = 8.
- Transpose PSUM writes must start bank-aligned (sub-bank offsets hang
  the HW), so V' transposes use one [128,64] tile each, tags alternating.
- ctx^T ([65, qw] = 64 dims + denominator row) goes out untransposed in
  f32; the host divides by the denominator row, transposes, un-permutes.
"""
import sys

if "/opt/trn_rl_repo" not in sys.path:
    sys.path.insert(0, "/opt/trn_rl_repo")

import numpy as np
import ml_dtypes

B, S, D = 2, 2048, 1024
H = 16
HD = 64
NCORES = 8
HPC = H // (NCORES // B)     # heads per core = 4
NG = HPC // 2                # head-pair groups per core = 2
KC = S // 128                # 16 key chunks

_CACHE = {}


def _qchunks(Mp):
    out, off = [], 0
    while off < Mp:
        w = min(512, Mp - off)
        out.append((off, w))
        off += w
    return out


def _build(Mp):
    import concourse.tile as tile
    from concourse import bacc, mybir
    from concourse.masks import make_identity
    from concourse.tile import add_dep_helper

    F32 = mybir.dt.float32
    BF16 = mybir.dt.bfloat16
    AF = mybir.ActivationFunctionType

    nc = bacc.Bacc(None, target_bir_lowering=False, debug=False)

    # X^T packed as one tile: col = sc*4096 + dc*512 + j (sc = 512-wide
    # s-chunk, dc = D chunk of 128) so every DMA is a contiguous range
    # and every matmul rhs slice [128, 512] is contiguous
    xt = nc.declare_dram_parameter("xt", [128, 8 * S], BF16, isOutput=False)
    wq = nc.declare_dram_parameter("wq", [128, NG * 8 * 128], BF16, isOutput=False)
    wk = nc.declare_dram_parameter("wk", [128, NG * 8 * 128], BF16, isOutput=False)
    b4 = nc.declare_dram_parameter("b4", [128, 4], F32, isOutput=False)
    out = nc.declare_dram_parameter("out", [NG * 2 * 65, Mp], F32, isOutput=True)

    qchunks = _qchunks(Mp)

    with tile.TileContext(nc) as tc, \
         nc.allow_low_precision(reason="bf16 pipeline; validated vs fp32 "
                                "reference at rel tol 2e-2"):
        with tc.tile_pool(name="consts", bufs=1) as consts, \
             tc.tile_pool(name="big", bufs=1) as big, \
             tc.tile_pool(name="tmp", bufs=2) as tmp, \
             tc.tile_pool(name="expp", bufs=3) as expp, \
             tc.tile_pool(name="ep", bufs=2) as ep, \
             tc.tile_pool(name="ps_s", bufs=1, space="PSUM") as ps_s, \
             tc.tile_pool(name="ps_c", bufs=1, space="PSUM") as ps_c, \
             tc.tile_pool(name="ps_q", bufs=1, space="PSUM") as ps_q:

            xt_t = consts.tile([128, 8 * S], BF16, name="xt_t")
            wq_t = consts.tile([128, NG * 8 * 128], BF16, name="wq_t")
            wk_t = consts.tile([128, NG * 8 * 128], BF16, name="wk_t")
            b4_t = consts.tile([128, 4], F32, name="b4_t")

            # pre-ramp the PE during the DMA window: ~14 dependency-free
            # matmuls bring the clock from the low p-state to full speed
            # before the first real projection matmul issues (scratch is
            # memset on the otherwise-idle vector engine so nothing waits
            # on the DMA queues)
            scr = consts.tile([128, 512], BF16, name="scr")
            nc.vector.memset(scr, 0.0)
            ramp_ps = ps_c.tile([128, 512], F32, tag="cA", name="ramp_ps")
            for _ in range(14):
                nc.tensor.matmul(ramp_ps, scr[:, 0:128], scr,
                                 start=True, stop=True)
            scr1 = consts.tile([1, 1], F32, name="scr1")
            nc.vector.tensor_copy(scr1, ramp_ps[0:1, 0:1])

            # startup DMA: wk-g0 first (Kproj g0 runs first), then ALL X
            # (contiguous 0.5MB transfers, halves split across the
            # sync/gpsimd queues so Kproj is never DMA-starved), then the
            # weights needed later.
            nc.gpsimd.dma_start(out=b4_t, in_=b4[:, :])
            nc.sync.dma_start(out=wk_t[:, 0:1024], in_=wk[:, 0:1024])
            for cc in range(4):
                base = cc * 4096
                nc.sync.dma_start(out=xt_t[:, base:base + 2048],
                                  in_=xt[:, base:base + 2048])
                nc.gpsimd.dma_start(out=xt_t[:, base + 2048:base + 4096],
                                    in_=xt[:, base + 2048:base + 4096])
            nc.sync.dma_start(out=wq_t[:, 0:1024], in_=wq[:, 0:1024])
            nc.gpsimd.dma_start(out=wk_t[:, 1024:2048], in_=wk[:, 1024:2048])
            nc.sync.dma_start(out=wq_t[:, 1024:2048], in_=wq[:, 1024:2048])

            def xts(dc, ssl):
                assert ssl.start % 512 == 0 and ssl.stop == ssl.start + 512
                base = (ssl.start // 512) * 4096 + dc * 512
                return xt_t[:, base:base + 512]

            bq_t = [b4_t[:, g:g + 1] for g in range(NG)]
            bk_t = [b4_t[:, 2 + g:3 + g] for g in range(NG)]

            identb = consts.tile([128, 128], BF16)
            make_identity(nc, identb)

            # persistent activations (bf16):
            # qt[g]: Q^T + bq, both heads packed on partitions
            # tk[g]: K^T + bk (linear, for V' = Q+K)
            # te[g]: e^(K^T + bk)   kt[g]: softplus = ln(te + 1)
            # vp[h]: V' chunks [128 keys, 64 dims + ones col] x 16
            qt = [big.tile([128, S], BF16, name=f"qt{g}") for g in range(NG)]
            tk = [big.tile([128, S], BF16, name=f"tk{g}") for g in range(NG)]
            te = [big.tile([128, S], BF16, name=f"te{g}") for g in range(NG)]
            kt = [big.tile([128, S], BF16, name=f"kt{g}") for g in range(NG)]
            vp = [big.tile([128, KC * 65], BF16, name=f"vp{h}")
                  for h in range(HPC)]
            for h in range(HPC):
                nc.gpsimd.memset(
                    vp[h].rearrange("p (k o) -> p k o", o=65)[:, :, 64:65],
                    1.0)

            te_insts = {0: [], 1: []}
            ln_insts = {}

            def kproj_pair(g, scp, tag):
                pk2 = ps_s.tile([128, 1024], F32, tag=tag,
                                name=f"pk{g}_{scp}")
                for lsc in range(2):
                    sc = scp * 2 + lsc
                    ssl = slice(sc * 512, (sc + 1) * 512)
                    psl = slice(lsc * 512, (lsc + 1) * 512)
                    for dc in range(8):
                        nc.tensor.matmul(
                            pk2[:, psl],
                            wk_t[:, g * 1024 + dc * 128:g * 1024 + (dc + 1) * 128],
                            xts(dc, ssl), start=(dc == 0), stop=(dc == 7))
                csl = slice(scp * 1024, (scp + 1) * 1024)
                nc.vector.tensor_scalar_add(tk[g][:, csl], pk2, bk_t[g])
                tei = nc.scalar.activation(out=te[g][:, csl], in_=pk2,
                                           func=AF.Exp, bias=bk_t[g])
                te_insts[g].append(tei)

            def kproj_chunk(g, sc, tag):
                # single 512-chunk flavor (used as attention fill)
                pk1 = ps_q.tile([128, 512], F32, tag=tag, name=f"pk{g}_{sc}c")
                ssl = slice(sc * 512, (sc + 1) * 512)
                for dc in range(8):
                    nc.tensor.matmul(
                        pk1,
                        wk_t[:, g * 1024 + dc * 128:g * 1024 + (dc + 1) * 128],
                        xts(dc, ssl), start=(dc == 0), stop=(dc == 7))
                nc.vector.tensor_scalar_add(tk[g][:, ssl], pk1, bk_t[g])
                tei = nc.scalar.activation(out=te[g][:, ssl], in_=pk1,
                                           func=AF.Exp, bias=bk_t[g])
                te_insts[g].append(tei)

            def emit_ln(g, extra_dep_insts=()):
                ln = nc.scalar.activation(out=kt[g], in_=te[g],
                                          func=AF.Ln, bias=1.0)
                for tei in te_insts[g]:
                    add_dep_helper(ln.ins, tei.ins, False,
                                   f"Ln g{g} after its te Exps")
                for di in extra_dep_insts:
                    add_dep_helper(ln.ins, di.ins, False,
                                   f"Ln g{g} table phase order")
                ln_insts[g] = ln

            def qproj_chunk(g, sc, tag):
                pq = ps_q.tile([128, 512], F32, tag=tag, name=f"pq{g}_{sc}")
                ssl = slice(sc * 512, (sc + 1) * 512)
                for dc in range(8):
                    nc.tensor.matmul(
                        pq,
                        wq_t[:, g * 1024 + dc * 128:g * 1024 + (dc + 1) * 128],
                        xts(dc, ssl), start=(dc == 0), stop=(dc == 7))
                nc.vector.tensor_scalar_add(qt[g][:, ssl], pq, bq_t[g])

            def vtrans_chunk(g, sc):
                # transpose PSUM writes must start bank-aligned, so one
                # [128,64] tile each; tags alternate so the WAR-on-copy
                # serialization pipelines
                ssl = slice(sc * 512, (sc + 1) * 512)
                vts = tmp.tile([128, 512], BF16, tag="vts",
                               name=f"vts{g}_{sc}")
                nc.vector.tensor_add(vts, qt[g][:, ssl], tk[g][:, ssl])
                for jj in range(4):
                    for hh in range(2):
                        h = g * 2 + hh
                        hsl = slice(hh * 64, (hh + 1) * 64)
                        j = sc * 4 + jj
                        pv = ps_q.tile([128, 64], BF16,
                                       tag="qA" if (jj * 2 + hh) % 2 == 0
                                       else "qB",
                                       name=f"pv{g}_{hh}_{j}")
                        nc.tensor.transpose(pv,
                                            vts[hsl, jj * 128:(jj + 1) * 128],
                                            identb[hsl, hsl])
                        nc.vector.tensor_copy(vp[h][:, j * 65:j * 65 + 64], pv)

            # ---- head: Kproj(g0) -> te/Ln(g0) -> Qproj(g0,c0); the first
            # scores supertile needs only kt, qt c0 -- V'(c0) moves into
            # the first fill slot (ctx lags one supertile anyway) ----
            kproj_pair(0, 0, "sA")
            kproj_pair(0, 1, "sB")
            emit_ln(0)
            qproj_chunk(0, 0, "qA")

            # ---- fill units: remaining projection work, emitted into the
            # attention stream one unit per supertile boundary. The tuned
            # placement assumes >= 2 full 512-query chunks per group; for
            # degenerate masks (tiny Mp) emit everything up front. ----
            ln_g1_extra = []   # attn-g0 qc0 exps, filled during emission
            if Mp >= 1024:
                fills = {
                    (0, 0): [lambda: vtrans_chunk(0, 0),
                             lambda: qproj_chunk(0, 1, "qB"),
                             lambda: vtrans_chunk(0, 1),
                             lambda: qproj_chunk(0, 2, "qA"),
                             lambda: vtrans_chunk(0, 2),
                             lambda: qproj_chunk(0, 3, "qB"),
                             lambda: vtrans_chunk(0, 3),
                             lambda: kproj_chunk(1, 0, "qA")],
                    (0, 1): [lambda: kproj_chunk(1, 1, "qB"),
                             lambda: kproj_chunk(1, 2, "qA"),
                             lambda: kproj_chunk(1, 3, "qB"),
                             lambda: emit_ln(1, extra_dep_insts=ln_g1_extra),
                             lambda: qproj_chunk(1, 0, "qA"),
                             lambda: vtrans_chunk(1, 0)],
                    (1, 0): [lambda: qproj_chunk(1, 1, "qB"),
                             lambda: vtrans_chunk(1, 1),
                             lambda: qproj_chunk(1, 2, "qA"),
                             lambda: vtrans_chunk(1, 2),
                             lambda: qproj_chunk(1, 3, "qB"),
                             lambda: vtrans_chunk(1, 3)],
                }
            else:
                fills = {}
                vtrans_chunk(0, 0)
                for sc in range(1, 4):
                    qproj_chunk(0, sc, "qB" if sc % 2 else "qA")
                    vtrans_chunk(0, sc)
                kproj_pair(1, 0, "sA")
                kproj_pair(1, 1, "sB")
                emit_ln(1)
                for sc in range(4):
                    qproj_chunk(1, sc, "qB" if sc % 2 else "qA")
                    vtrans_chunk(1, sc)

            # ---- attention: one continuous software pipeline over
            # (group, query-chunk, supertile); ctx lags scores by one
            # supertile so the in-order PE queue never stalls on an exp ----
            pend_ctx = []

            def attn():
                units = [(g,) + qc for g in range(NG) for qc in qchunks]
                for g, qoff, qw in units:
                    ns = 1024 // qw
                    qsl = slice(qoff, qoff + qw)
                    cA = ps_c.tile([65, qw], F32, tag="cA",
                                   name=f"cA{g}_{qoff}")
                    cB = ps_c.tile([65, qw], F32, tag="cB",
                                   name=f"cB{g}_{qoff}")
                    qci = _qchunks(Mp).index((qoff, qw))
                    fill = fills.get((g, qci), [])
                    for sti in range(KC // ns):
                        kc0 = sti * ns
                        sA = ps_s.tile([128, 1024], F32, tag="sA",
                                       name=f"sA{g}_{qoff}_{kc0}")
                        sB = ps_s.tile([128, 1024], F32, tag="sB",
                                       name=f"sB{g}_{qoff}_{kc0}")
                        for kk in range(ns):
                            kc = kc0 + kk
                            osl = slice(kk * qw, (kk + 1) * qw)
                            ksl = slice(kc * 128, (kc + 1) * 128)
                            nc.tensor.matmul(sA[:, osl], kt[g][0:64, ksl],
                                             qt[g][0:64, qsl],
                                             start=True, stop=True)
                            nc.tensor.matmul(sB[:, osl], kt[g][64:128, ksl],
                                             qt[g][64:128, qsl],
                                             start=True, stop=True)
                        eA = expp.tile([128, 1024], BF16, tag="eA",
                                       name=f"eA{g}_{qoff}_{kc0}")
                        eiA = nc.scalar.activation(out=eA, in_=sA,
                                                   func=AF.Exp, scale=0.125)
                        eB = expp.tile([128, 1024], BF16, tag="eB",
                                       name=f"eB{g}_{qoff}_{kc0}")
                        eiB = nc.scalar.activation(out=eB, in_=sB,
                                                   func=AF.Exp, scale=0.125)
                        for ei in (eiA, eiB):
                            add_dep_helper(ei.ins, ln_insts[g].ins, False,
                                           "attn Exp after its Ln")
                        if g == 0 and qci == 0:
                            ln_g1_extra.extend([eiA, eiB])

                        def emit_ctx(g=g, cA=cA, cB=cB, eA=eA, eB=eB,
                                     kc0=kc0, ns=ns, qw=qw):
                            for kk in range(ns):
                                kc = kc0 + kk
                                osl = slice(kk * qw, (kk + 1) * qw)
                                nc.tensor.matmul(
                                    cA, vp[g * 2][:, kc * 65:(kc + 1) * 65],
                                    eA[:, osl],
                                    start=(kc == 0), stop=(kc == KC - 1))
                                nc.tensor.matmul(
                                    cB, vp[g * 2 + 1][:, kc * 65:(kc + 1) * 65],
                                    eB[:, osl],
                                    start=(kc == 0), stop=(kc == KC - 1))
                        pend_ctx.append(emit_ctx)

                        if fill:
                            fill.pop(0)()
                        if len(pend_ctx) > 1:
                            pend_ctx.pop(0)()
                    # qc epilogue: flush the last ctx, then DVE copy + DMA
                    # (DVE/DMA queues wait on sems; the PE moves on)
                    pend_ctx.pop(0)()
                    csA = ep.tile([65, qw], F32, tag="csA",
                                  name=f"csA{g}_{qoff}")
                    nc.vector.tensor_copy(csA, cA)
                    csB = ep.tile([65, qw], F32, tag="csB",
                                  name=f"csB{g}_{qoff}")
                    nc.vector.tensor_copy(csB, cB)
                    r0 = (g * 2) * 65
                    r1 = (g * 2 + 1) * 65
                    nc.sync.dma_start(out=out[r0:r0 + 65, qsl], in_=csA)
                    nc.gpsimd.dma_start(out=out[r1:r1 + 65, qsl], in_=csB)

            attn()

    nc.finalize()
    return nc


def _get_nc(Mp):
    key = ("nc", Mp)
    if key not in _CACHE:
        _CACHE[key] = _build(Mp)
    return _CACHE[key]


def _shard_inputs(hidden_states, attention_mask, Wq, bq, Wk, bk):
    bf16 = ml_dtypes.bfloat16
    hs = np.asarray(hidden_states, dtype=np.float32)
    am = np.asarray(attention_mask)
    Wq = np.asarray(Wq, dtype=np.float32)
    Wk = np.asarray(Wk, dtype=np.float32)
    bq = np.asarray(bq, dtype=np.float32)
    bk = np.asarray(bk, dtype=np.float32)

    # unmasked queries first; masked-query outputs are uniform-softmax
    # averages computed on host
    perms = [np.argsort(am[b] == 0, kind="stable") for b in range(B)]
    Ms = [int((am[b] != 0).sum()) for b in range(B)]
    Mp = max(256, -(-max(Ms) // 128) * 128)
    Mp = min(Mp, S)
    meanv = [hs[b].mean(axis=0) @ (Wq + Wk) + bq + bk for b in range(B)]

    xts = []
    for b in range(B):
        xp = np.ascontiguousarray(hs[b][perms[b]].T).astype(bf16)  # [D, S]
        # device layout: col = sc*4096 + dc*512 + j
        xts.append(np.ascontiguousarray(
            xp.reshape(8, 128, 4, 512).transpose(1, 2, 0, 3)
              .reshape(128, 8 * S)))

    in_maps = []
    for c in range(NCORES):
        b = c // (NCORES // B)
        hg = c % (NCORES // B)
        cols = slice(hg * 2 * 128, (hg + 1) * 2 * 128)

        def _tile_w(W):
            # [128, g*1024 + dc*128 + j] = W[dc*128 + p, cols[g*128 + j]]
            a = W[:, cols].reshape(8, 128, NG, 128).transpose(1, 2, 0, 3)
            return np.ascontiguousarray(a.reshape(128, NG * 8 * 128)).astype(bf16)

        bqs, bks = bq[cols], bk[cols]
        b4 = np.ascontiguousarray(np.stack(
            [bqs[0:128], bqs[128:256], bks[0:128], bks[128:256]],
            axis=1).astype(np.float32))
        in_maps.append({
            "xt": xts[b],
            "wq": _tile_w(Wq),
            "wk": _tile_w(Wk),
            "b4": b4,
        })
    _CACHE["host"] = {"perms": perms, "Ms": Ms, "Mp": Mp, "meanv": meanv}
    return in_maps


def _gather(results):
    ctx = _CACHE["host"]
    perms, Ms, Mp, meanv = ctx["perms"], ctx["Ms"], ctx["Mp"], ctx["meanv"]
    full = np.empty((B, S, D), dtype=np.float32)
    for b in range(B):
        full[b, perms[b][Ms[b]:], :] = meanv[b][None, :]
    for c in range(NCORES):
        b = c // (NCORES // B)
        hg = c % (NCORES // B)
        r = results[c]["out"]          # [NG*2*65, Mp]
        M = Ms[b]
        rows = perms[b][:M]
        for g in range(NG):
            for hh in range(2):
                blk = r[(g * 2 + hh) * 65:(g * 2 + hh) * 65 + 65, :M]
                col = hg * 256 + (g * 2 + hh) * 64
                full[b, rows, col:col + 64] = (blk[0:64] / blk[64:65]).T
    return full


def run_sharded(in_maps, **kw):
    from concourse.bass_utils import run_bass_kernel_spmd
    nc = _get_nc(_CACHE["host"]["Mp"])
    return run_bass_kernel_spmd(nc, in_maps, list(range(NCORES)), **kw)


def kernel(hidden_states, attention_mask, Wq, bq, Wk, bk):
    in_maps = _shard_inputs(hidden_states, attention_mask, Wq, bq, Wk, bk)
    res = run_sharded(in_maps)
    return _gather(res.results)


# revision 17
# speedup vs baseline: 1.0334x; 1.0334x over previous
"""Bass/Tile TRN2 kernel for nn_BertSelfAttention2 (B=2, S=2048, D=1024, H=16).

Sharding: 8 cores = 2 (batch) x 4 (head groups of 4 heads). Each core
computes Q/K projections for its 4 heads (2 packed pairs), the modified
attention (kt = softplus(k), v = q + k, mask on the query axis), and
writes its ctx^T slices; the host does the final divide + transpose.

v5 design (query compaction + full PE/ACT software pipeline):
- The mask hits only the QUERY axis; a masked query's softmax is uniform
  so its output is mean_k V[k] -- identical for every masked query of a
  (batch, head). The host permutes positions so unmasked queries come
  first, the device computes attention only for the first Mp (= padded
  unmasked count) query positions, and the host fills masked rows with
  mean(V) = mean_k X[k] @ (Wq+Wk) + bq + bk computed in fp32 on CPU.
  Key-axis work (projections, softplus, V') stays full-length: V = Q + K
  needs Q at every position. For the harness inputs Mp = 1152 of 2048.
- No mask machinery on device: pad queries in [M_b, Mp) are real (masked)
  positions whose outputs the host discards.
- Scores matmuls use 64-row operands (row group h0/h64 per head).
- The ACT engine is the bottleneck once attention starts (~1150ns per
  [128,1024] exp); the schedule keeps it saturated: Kproj(g0) -> te/Ln(g0)
  -> Qproj(g0,c0)+V' -> attention starts at ~t20. ALL remaining
  projection work (Qproj g0 c1-3, Kproj/Qproj/V' of g1) is emitted as
  fill units, one per supertile boundary, inside the attention stream.
  ctx matmuls lag one supertile behind scores so the in-order PE queue
  never stalls waiting for an exp. 5 ACT table loads total
  (Exp | Ln g0 | Exp: attn-g0 + te-g1 | Ln g1 | Exp: attn-g1).
- PSUM tags: sA,sB = [128,1024] f32 scores supertiles (2 banks each,
  double-buffered across supers), cA,cB = ctx accumulators (1 bank),
  qA,qB = projection chunk accumulators + V' transposes (1 bank) = 8.
- Transpose PSUM writes must start bank-aligned (sub-bank offsets hang
  the HW), so V' transposes use one [128,64] tile each, tags alternating.
- ctx^T ([65, qw] = 64 dims + denominator row) goes out untransposed in
  f32; the host divides by the denominator row, transposes, un-permutes.
"""
import sys

if "/opt/trn_rl_repo" not in sys.path:
    sys.path.insert(0, "/opt/trn_rl_repo")

import numpy as np
import ml_dtypes

B, S, D = 2, 2048, 1024
H = 16
HD = 64
NCORES = 8
HPC = H // (NCORES // B)     # heads per core = 4
NG = HPC // 2                # head-pair groups per core = 2
KC = S // 128                # 16 key chunks

_CACHE = {}


def _qchunks(Mp):
    out, off = [], 0
    while off < Mp:
        w = min(512, Mp - off)
        out.append((off, w))
        off += w
    return out


def _build(Mp):
    import concourse.tile as tile
    from concourse import bacc, mybir
    from concourse.masks import make_identity
    from concourse.tile import add_dep_helper

    F32 = mybir.dt.float32
    BF16 = mybir.dt.bfloat16
    AF = mybir.ActivationFunctionType

    nc = bacc.Bacc(None, target_bir_lowering=False, debug=False)

    # X^T packed as one tile: col = sc*4096 + dc*512 + j (sc = 512-wide
    # s-chunk, dc = D chunk of 128) so every DMA is a contiguous range
    # and every matmul rhs slice [128, 512] is contiguous
    xt = nc.declare_dram_parameter("xt", [128, 8 * S], BF16, isOutput=False)
    wq = nc.declare_dram_parameter("wq", [128, NG * 8 * 128], BF16, isOutput=False)
    wk = nc.declare_dram_parameter("wk", [128, NG * 8 * 128], BF16, isOutput=False)
    b4 = nc.declare_dram_parameter("b4", [128, 4], F32, isOutput=False)
    out = nc.declare_dram_parameter("out", [NG * 2 * 65, Mp], F32, isOutput=True)

    qchunks = _qchunks(Mp)

    with tile.TileContext(nc) as tc, \
         nc.allow_low_precision(reason="bf16 pipeline; validated vs fp32 "
                                "reference at rel tol 2e-2"):
        with tc.tile_pool(name="consts", bufs=1) as consts, \
             tc.tile_pool(name="big", bufs=1) as big, \
             tc.tile_pool(name="tmp", bufs=3) as tmp, \
             tc.tile_pool(name="expp", bufs=6) as expp, \
             tc.tile_pool(name="ep", bufs=4) as ep, \
             tc.tile_pool(name="ps_s", bufs=1, space="PSUM") as ps_s, \
             tc.tile_pool(name="ps_c", bufs=1, space="PSUM") as ps_c, \
             tc.tile_pool(name="ps_q", bufs=1, space="PSUM") as ps_q:

            xt_t = consts.tile([128, 8 * S], BF16, name="xt_t")
            wq_t = consts.tile([128, NG * 8 * 128], BF16, name="wq_t")
            wk_t = consts.tile([128, NG * 8 * 128], BF16, name="wk_t")
            b4_t = consts.tile([128, 4], F32, name="b4_t")

            # pre-ramp the PE during the DMA window: ~14 dependency-free
            # matmuls bring the clock from the low p-state to full speed
            # before the first real projection matmul issues (scratch is
            # memset on the otherwise-idle vector engine so nothing waits
            # on the DMA queues)
            scr = consts.tile([128, 512], BF16, name="scr")
            nc.vector.memset(scr, 0.0)
            ramp_ps = ps_c.tile([128, 512], F32, tag="cA", name="ramp_ps")
            for _ in range(14):
                nc.tensor.matmul(ramp_ps, scr[:, 0:128], scr,
                                 start=True, stop=True)
            scr1 = consts.tile([1, 1], F32, name="scr1")
            nc.vector.tensor_copy(scr1, ramp_ps[0:1, 0:1])

            # startup DMA: wk-g0 first (Kproj g0 runs first), then ALL X
            # (contiguous 0.5MB transfers, halves split across the
            # sync/gpsimd queues so Kproj is never DMA-starved), then the
            # weights needed later.
            nc.gpsimd.dma_start(out=b4_t, in_=b4[:, :])
            nc.sync.dma_start(out=wk_t[:, 0:1024], in_=wk[:, 0:1024])
            for cc in range(4):
                base = cc * 4096
                nc.sync.dma_start(out=xt_t[:, base:base + 2048],
                                  in_=xt[:, base:base + 2048])
                nc.gpsimd.dma_start(out=xt_t[:, base + 2048:base + 4096],
                                    in_=xt[:, base + 2048:base + 4096])
            nc.sync.dma_start(out=wq_t[:, 0:1024], in_=wq[:, 0:1024])
            nc.gpsimd.dma_start(out=wk_t[:, 1024:2048], in_=wk[:, 1024:2048])
            nc.sync.dma_start(out=wq_t[:, 1024:2048], in_=wq[:, 1024:2048])

            def xts(dc, ssl):
                assert ssl.start % 512 == 0 and ssl.stop == ssl.start + 512
                base = (ssl.start // 512) * 4096 + dc * 512
                return xt_t[:, base:base + 512]

            bq_t = [b4_t[:, g:g + 1] for g in range(NG)]
            bk_t = [b4_t[:, 2 + g:3 + g] for g in range(NG)]

            identb = consts.tile([128, 128], BF16)
            make_identity(nc, identb)

            # persistent activations (bf16):
            # qt[g]: Q^T + bq, both heads packed on partitions
            # tk[g]: K^T + bk (linear, for V' = Q+K)
            # te[g]: e^(K^T + bk)   kt[g]: softplus = ln(te + 1)
            # vp[h]: V' chunks [128 keys, 64 dims + ones col] x 16
            qt = [big.tile([128, S], BF16, name=f"qt{g}") for g in range(NG)]
            tk = [big.tile([128, S], BF16, name=f"tk{g}") for g in range(NG)]
            te = [big.tile([128, S], BF16, name=f"te{g}") for g in range(NG)]
            kt = [big.tile([128, S], BF16, name=f"kt{g}") for g in range(NG)]
            vp = [big.tile([128, KC * 65], BF16, name=f"vp{h}")
                  for h in range(HPC)]
            for h in range(HPC):
                nc.gpsimd.memset(
                    vp[h].rearrange("p (k o) -> p k o", o=65)[:, :, 64:65],
                    1.0)

            te_insts = {0: [], 1: []}
            ln_insts = {}

            def kproj_pair(g, scp, tag):
                pk2 = ps_s.tile([128, 1024], F32, tag=tag,
                                name=f"pk{g}_{scp}")
                for lsc in range(2):
                    sc = scp * 2 + lsc
                    ssl = slice(sc * 512, (sc + 1) * 512)
                    psl = slice(lsc * 512, (lsc + 1) * 512)
                    for dc in range(8):
                        nc.tensor.matmul(
                            pk2[:, psl],
                            wk_t[:, g * 1024 + dc * 128:g * 1024 + (dc + 1) * 128],
                            xts(dc, ssl), start=(dc == 0), stop=(dc == 7))
                csl = slice(scp * 1024, (scp + 1) * 1024)
                nc.vector.tensor_scalar_add(tk[g][:, csl], pk2, bk_t[g])
                tei = nc.scalar.activation(out=te[g][:, csl], in_=pk2,
                                           func=AF.Exp, bias=bk_t[g])
                te_insts[g].append(tei)

            def kproj_chunk(g, sc, tag):
                # single 512-chunk flavor (used as attention fill)
                pk1 = ps_q.tile([128, 512], F32, tag=tag, name=f"pk{g}_{sc}c")
                ssl = slice(sc * 512, (sc + 1) * 512)
                for dc in range(8):
                    nc.tensor.matmul(
                        pk1,
                        wk_t[:, g * 1024 + dc * 128:g * 1024 + (dc + 1) * 128],
                        xts(dc, ssl), start=(dc == 0), stop=(dc == 7))
                nc.vector.tensor_scalar_add(tk[g][:, ssl], pk1, bk_t[g])
                tei = nc.scalar.activation(out=te[g][:, ssl], in_=pk1,
                                           func=AF.Exp, bias=bk_t[g])
                te_insts[g].append(tei)

            def emit_ln(g, extra_dep_insts=()):
                ln = nc.scalar.activation(out=kt[g], in_=te[g],
                                          func=AF.Ln, bias=1.0)
                for tei in te_insts[g]:
                    add_dep_helper(ln.ins, tei.ins, False,
                                   f"Ln g{g} after its te Exps")
                for di in extra_dep_insts:
                    add_dep_helper(ln.ins, di.ins, False,
                                   f"Ln g{g} table phase order")
                ln_insts[g] = ln

            def qproj_chunk(g, sc, tag):
                pq = ps_q.tile([128, 512], F32, tag=tag, name=f"pq{g}_{sc}")
                ssl = slice(sc * 512, (sc + 1) * 512)
                for dc in range(8):
                    nc.tensor.matmul(
                        pq,
                        wq_t[:, g * 1024 + dc * 128:g * 1024 + (dc + 1) * 128],
                        xts(dc, ssl), start=(dc == 0), stop=(dc == 7))
                nc.vector.tensor_scalar_add(qt[g][:, ssl], pq, bq_t[g])

            def vtrans_chunk(g, sc):
                # transpose PSUM writes must start bank-aligned, so one
                # [128,64] tile each; tags alternate so the WAR-on-copy
                # serialization pipelines
                ssl = slice(sc * 512, (sc + 1) * 512)
                vts = tmp.tile([128, 512], BF16, tag="vts",
                               name=f"vts{g}_{sc}")
                nc.vector.tensor_add(vts, qt[g][:, ssl], tk[g][:, ssl])
                for jj in range(4):
                    for hh in range(2):
                        h = g * 2 + hh
                        hsl = slice(hh * 64, (hh + 1) * 64)
                        j = sc * 4 + jj
                        pv = ps_q.tile([128, 64], BF16,
                                       tag="qA" if (jj * 2 + hh) % 2 == 0
                                       else "qB",
                                       name=f"pv{g}_{hh}_{j}")
                        nc.tensor.transpose(pv,
                                            vts[hsl, jj * 128:(jj + 1) * 128],
                                            identb[hsl, hsl])
                        nc.vector.tensor_copy(vp[h][:, j * 65:j * 65 + 64], pv)

            # ---- head: Kproj(g0) -> te/Ln(g0) -> Qproj(g0,c0); the first
            # scores supertile needs only kt, qt c0 -- V'(c0) moves into
            # the first fill slot (ctx lags one supertile anyway) ----
            kproj_pair(0, 0, "sA")
            kproj_pair(0, 1, "sB")
            emit_ln(0)
            qproj_chunk(0, 0, "qA")

            # ---- fill units: remaining projection work, emitted into the
            # attention stream one unit per supertile boundary. The tuned
            # placement assumes >= 2 full 512-query chunks per group; for
            # degenerate masks (tiny Mp) emit everything up front. ----
            ln_g1_extra = []   # attn-g0 qc0 exps, filled during emission
            if Mp >= 1024:
                fills = {
                    (0, 0): [lambda: vtrans_chunk(0, 0),
                             lambda: qproj_chunk(0, 1, "qB"),
                             lambda: vtrans_chunk(0, 1),
                             lambda: qproj_chunk(0, 2, "qA"),
                             lambda: vtrans_chunk(0, 2),
                             lambda: qproj_chunk(0, 3, "qB"),
                             lambda: vtrans_chunk(0, 3),
                             lambda: kproj_chunk(1, 0, "qA")],
                    (0, 1): [lambda: kproj_chunk(1, 1, "qB"),
                             lambda: kproj_chunk(1, 2, "qA"),
                             lambda: kproj_chunk(1, 3, "qB"),
                             lambda: emit_ln(1, extra_dep_insts=ln_g1_extra),
                             lambda: qproj_chunk(1, 0, "qA"),
                             lambda: vtrans_chunk(1, 0)],
                    (1, 0): [lambda: qproj_chunk(1, 1, "qB"),
                             lambda: vtrans_chunk(1, 1),
                             lambda: qproj_chunk(1, 2, "qA"),
                             lambda: vtrans_chunk(1, 2),
                             lambda: qproj_chunk(1, 3, "qB"),
                             lambda: vtrans_chunk(1, 3)],
                }
            else:
                fills = {}
                vtrans_chunk(0, 0)
                for sc in range(1, 4):
                    qproj_chunk(0, sc, "qB" if sc % 2 else "qA")
                    vtrans_chunk(0, sc)
                kproj_pair(1, 0, "sA")
                kproj_pair(1, 1, "sB")
                emit_ln(1)
                for sc in range(4):
                    qproj_chunk(1, sc, "qB" if sc % 2 else "qA")
                    vtrans_chunk(1, sc)

            # ---- attention: one continuous software pipeline over
            # (group, query-chunk, supertile); ctx lags scores by one
            # supertile so the in-order PE queue never stalls on an exp ----
            pend_ctx = []

            def attn():
                units = [(g,) + qc for g in range(NG) for qc in qchunks]
                for g, qoff, qw in units:
                    ns = 1024 // qw
                    qsl = slice(qoff, qoff + qw)
                    cA = ps_c.tile([65, qw], F32, tag="cA",
                                   name=f"cA{g}_{qoff}")
                    cB = ps_c.tile([65, qw], F32, tag="cB",
                                   name=f"cB{g}_{qoff}")
                    qci = _qchunks(Mp).index((qoff, qw))
                    fill = fills.get((g, qci), [])
                    for sti in range(KC // ns):
                        kc0 = sti * ns
                        sA = ps_s.tile([128, 1024], F32, tag="sA",
                                       name=f"sA{g}_{qoff}_{kc0}")
                        sB = ps_s.tile([128, 1024], F32, tag="sB",
                                       name=f"sB{g}_{qoff}_{kc0}")
                        for kk in range(ns):
                            kc = kc0 + kk
                            osl = slice(kk * qw, (kk + 1) * qw)
                            ksl = slice(kc * 128, (kc + 1) * 128)
                            nc.tensor.matmul(sA[:, osl], kt[g][0:64, ksl],
                                             qt[g][0:64, qsl],
                                             start=True, stop=True)
                            nc.tensor.matmul(sB[:, osl], kt[g][64:128, ksl],
                                             qt[g][64:128, qsl],
                                             start=True, stop=True)
                        eA = expp.tile([128, 1024], BF16, tag="eA",
                                       name=f"eA{g}_{qoff}_{kc0}")
                        eiA = nc.scalar.activation(out=eA, in_=sA,
                                                   func=AF.Exp, scale=0.125)
                        eB = expp.tile([128, 1024], BF16, tag="eB",
                                       name=f"eB{g}_{qoff}_{kc0}")
                        eiB = nc.scalar.activation(out=eB, in_=sB,
                                                   func=AF.Exp, scale=0.125)
                        for ei in (eiA, eiB):
                            add_dep_helper(ei.ins, ln_insts[g].ins, False,
                                           "attn Exp after its Ln")
                        if g == 0 and qci == 0:
                            ln_g1_extra.extend([eiA, eiB])

                        def emit_ctx(g=g, cA=cA, cB=cB, eA=eA, eB=eB,
                                     kc0=kc0, ns=ns, qw=qw):
                            for kk in range(ns):
                                kc = kc0 + kk
                                osl = slice(kk * qw, (kk + 1) * qw)
                                nc.tensor.matmul(
                                    cA, vp[g * 2][:, kc * 65:(kc + 1) * 65],
                                    eA[:, osl],
                                    start=(kc == 0), stop=(kc == KC - 1))
                                nc.tensor.matmul(
                                    cB, vp[g * 2 + 1][:, kc * 65:(kc + 1) * 65],
                                    eB[:, osl],
                                    start=(kc == 0), stop=(kc == KC - 1))
                        pend_ctx.append(emit_ctx)

                        if fill:
                            fill.pop(0)()
                        if len(pend_ctx) > 1:
                            pend_ctx.pop(0)()
                    # qc epilogue: flush the last ctx, then DVE copy + DMA
                    # (DVE/DMA queues wait on sems; the PE moves on)
                    pend_ctx.pop(0)()
                    csA = ep.tile([65, qw], F32, tag="csA",
                                  name=f"csA{g}_{qoff}")
                    nc.vector.tensor_copy(csA, cA)
                    csB = ep.tile([65, qw], F32, tag="csB",
                                  name=f"csB{g}_{qoff}")
                    nc.vector.tensor_copy(csB, cB)
                    r0 = (g * 2) * 65
                    r1 = (g * 2 + 1) * 65
                    nc.sync.dma_start(out=out[r0:r0 + 65, qsl], in_=csA)
                    nc.gpsimd.dma_start(out=out[r1:r1 + 65, qsl], in_=csB)

            attn()

    nc.finalize()
    return nc


def _get_nc(Mp):
    key = ("nc", Mp)
    if key not in _CACHE:
        _CACHE[key] = _build(Mp)
    return _CACHE[key]


def _shard_inputs(hidden_states, attention_mask, Wq, bq, Wk, bk):
    bf16 = ml_dtypes.bfloat16
    hs = np.asarray(hidden_states, dtype=np.float32)
    am = np.asarray(attention_mask)
    Wq = np.asarray(Wq, dtype=np.float32)
    Wk = np.asarray(Wk, dtype=np.float32)
    bq = np.asarray(bq, dtype=np.float32)
    bk = np.asarray(bk, dtype=np.float32)

    # unmasked queries first; masked-query outputs are uniform-softmax
    # averages computed on host
    perms = [np.argsort(am[b] == 0, kind="stable") for b in range(B)]
    Ms = [int((am[b] != 0).sum()) for b in range(B)]
    Mp = max(256, -(-max(Ms) // 128) * 128)
    Mp = min(Mp, S)
    meanv = [hs[b].mean(axis=0) @ (Wq + Wk) + bq + bk for b in range(B)]

    xts = []
    for b in range(B):
        xp = np.ascontiguousarray(hs[b][perms[b]].T).astype(bf16)  # [D, S]
        # device layout: col = sc*4096 + dc*512 + j
        xts.append(np.ascontiguousarray(
            xp.reshape(8, 128, 4, 512).transpose(1, 2, 0, 3)
              .reshape(128, 8 * S)))

    in_maps = []
    for c in range(NCORES):
        b = c // (NCORES // B)
        hg = c % (NCORES // B)
        cols = slice(hg * 2 * 128, (hg + 1) * 2 * 128)

        def _tile_w(W):
            # [128, g*1024 + dc*128 + j] = W[dc*128 + p, cols[g*128 + j]]
            a = W[:, cols].reshape(8, 128, NG, 128).transpose(1, 2, 0, 3)
            return np.ascontiguousarray(a.reshape(128, NG * 8 * 128)).astype(bf16)

        bqs, bks = bq[cols], bk[cols]
        b4 = np.ascontiguousarray(np.stack(
            [bqs[0:128], bqs[128:256], bks[0:128], bks[128:256]],
            axis=1).astype(np.float32))
        in_maps.append({
            "xt": xts[b],
            "wq": _tile_w(Wq),
            "wk": _tile_w(Wk),
            "b4": b4,
        })
    _CACHE["host"] = {"perms": perms, "Ms": Ms, "Mp": Mp, "meanv": meanv}
    return in_maps


def _gather(results):
    ctx = _CACHE["host"]
    perms, Ms, Mp, meanv = ctx["perms"], ctx["Ms"], ctx["Mp"], ctx["meanv"]
    full = np.empty((B, S, D), dtype=np.float32)
    for b in range(B):
        full[b, perms[b][Ms[b]:], :] = meanv[b][None, :]
    for c in range(NCORES):
        b = c // (NCORES // B)
        hg = c % (NCORES // B)
        r = results[c]["out"]          # [NG*2*65, Mp]
        M = Ms[b]
        rows = perms[b][:M]
        for g in range(NG):
            for hh in range(2):
                blk = r[(g * 2 + hh) * 65:(g * 2 + hh) * 65 + 65, :M]
                col = hg * 256 + (g * 2 + hh) * 64
                full[b, rows, col:col + 64] = (blk[0:64] / blk[64:65]).T
    return full


def run_sharded(in_maps, **kw):
    from concourse.bass_utils import run_bass_kernel_spmd
    nc = _get_nc(_CACHE["host"]["Mp"])
    return run_bass_kernel_spmd(nc, in_maps, list(range(NCORES)), **kw)


def kernel(hidden_states, attention_mask, Wq, bq, Wk, bk):
    in_maps = _shard_inputs(hidden_states, attention_mask, Wq, bq, Wk, bk)
    res = run_sharded(in_maps)
    return _gather(res.results)
